# revision 1
# baseline (speedup 1.0000x reference)
"""CAB (channel-attention block) Trainium2 kernel.

Sharding: 8 cores = 4 batches x 2 H-halves. Each core computes its
[64, 128, 256] slice of the output. The q.kT contraction and the L2-norm
sums are AllReduced across the 2 cores sharing a batch (sequence-parallel).

Math folds used (all exact):
  - L2 normalize folds into S: attn_logits = S * temp / (||q|| ||k||^T),
    with S = q @ k^T computed on raw (unnormalized) q, k.
  - proj o (attn @ v) o dwconv_v o pwconv_v folds into a single 3x3 dense
    conv on input2 with data-dependent matrices
    G[dp] = (P @ A_blockdiag) @ (diag(wdv[:,dp]) @ Wv).
"""
import sys

sys.path.insert(0, "/opt/trn_rl_repo")

import numpy as np

import concourse.bacc as bacc
import concourse.bass as bass
import concourse.tile as tile
from concourse import mybir
from concourse.bass_utils import run_bass_kernel_spmd

F32 = mybir.dt.float32

B, C, H, W = 4, 64, 256, 256
HEADS = 8
HD = C // HEADS
EPS = 1e-12

HH = H // 2            # rows per core
R = W + 2              # padded row length
NR = HH + 4            # buffer rows: zero, halo, 128 data, halo, zero
NBUF = NR * R          # per-core padded input length (flattened)
P0 = 2 * R             # first output position (data row 0, col -1(pad))
NOUT = HH * R          # output span incl. per-row col pads

TAPS = [(dy, dx) for dy in (-1, 0, 1) for dx in (-1, 0, 1)]
# rhs offset of tap within a window that starts 259 cols before the chunk
TAP_OFF = [259 + dy * R + dx for dy, dx in TAPS]

SPAN1 = 2048           # pass-1 window span (multiple of 512)
ROWS2 = 8              # pass-2 window rows

_CACHE = {}


def _pad_positions(start, length):
    """Contiguous runs of pad columns (global col % R in {0, R-1}) within
    [start, start+length), as (offset_rel, run_len) with runs <= 2."""
    runs = []
    g = start
    end = start + length
    # pads occur at positions p with p % R == 0 or R-1; adjacent pairs.
    p = (start // R) * R - 1
    while p < end:
        for q in (p, p + 1):  # (row r col 257, row r+1 col 0) adjacent pair
            lo = max(q, start)
            hi = min(q + 1, end)
            if lo < hi:
                if runs and runs[-1][0] + runs[-1][1] == lo - start:
                    runs[-1] = (runs[-1][0], runs[-1][1] + (hi - lo))
                else:
                    runs.append((lo - start, hi - lo))
        p += R
    return runs


def build_module(mode="full"):
    nc = bacc.Bacc("TRN2", target_bir_lowering=False, debug=False, num_devices=8)

    x1 = nc.declare_dram_parameter("x1", [C, NBUF], F32, isOutput=False)
    x2 = nc.declare_dram_parameter("x2", [C, NBUF], F32, isOutput=False)
    lqkT = nc.declare_dram_parameter("lqkT", [128, 9 * C], F32, isOutput=False)
    wv9 = nc.declare_dram_parameter("wv9", [C, 9 * C], F32, isOutput=False)
    pT = nc.declare_dram_parameter("pT", [C, C], F32, isOutput=False)
    temp64 = nc.declare_dram_parameter("temp64", [C, 1], F32, isOutput=False)
    ident = nc.declare_dram_parameter("ident", [128, 128], F32, isOutput=False)
    mask64 = nc.declare_dram_parameter("mask64", [C, C], F32, isOutput=False)
    y = nc.declare_dram_parameter("y", [C, HH, W], F32, isOutput=True)

    with tile.TileContext(nc) as tc:
        _body(tc, nc, x1, x2, lqkT, wv9, pT, temp64, ident, mask64, y, mode)
    nc.compile()
    return nc


def _body(tc, nc, x1, x2, lqkT, wv9, pT, temp64, ident, mask64, y, mode="full"):
    mm = nc.tensor.matmul
    f = F32

    wpool = tc.alloc_tile_pool(name="weights", bufs=1)
    dram = tc.alloc_tile_pool(name="dram", bufs=1, space="DRAM")
    accp = tc.alloc_tile_pool(name="ps_acc", bufs=1, space=bass.MemorySpace.PSUM)
    persist = tc.alloc_tile_pool(name="persist", bufs=1)

    w_lqkT = wpool.tile([128, 9 * C], f)
    nc.gpsimd.dma_start(w_lqkT[:], lqkT[:])
    w_wv9 = wpool.tile([C, 9 * C], f)
    nc.gpsimd.dma_start(w_wv9[:], wv9[:])
    w_pT = wpool.tile([C, C], f)
    nc.gpsimd.dma_start(w_pT[:], pT[:])
    w_temp = wpool.tile([C, 1], f)
    nc.gpsimd.dma_start(w_temp[:], temp64[:])
    w_id = wpool.tile([128, 128], f)
    nc.gpsimd.dma_start(w_id[:], ident[:])
    w_mask = wpool.tile([C, C], f)
    nc.gpsimd.dma_start(w_mask[:], mask64[:])

    acc_ps = accp.tile([C, C], f)          # S accumulator (q.kT)
    qk2 = persist.tile([128, 1], f)        # running sum q^2 (top) / k^2 (bottom)
    nc.vector.memset(qk2[:], 0.0)

    # ---------------- pass 1: q,k conv -> transpose -> S, norms ----------
    n_sub_total = NOUT // 128
    sub_idx = 0
    with (
        tc.tile_pool(name="xw1", bufs=2) as xw1p,
        tc.tile_pool(name="qkwin", bufs=2) as qkwp,
        tc.tile_pool(name="trsb", bufs=3) as trsbp,
        tc.tile_pool(name="scratch", bufs=1) as scrp,
        tc.tile_pool(name="ps_conv", bufs=2, space=bass.MemorySpace.PSUM) as pcv,
        tc.tile_pool(name="ps_tr", bufs=2, space=bass.MemorySpace.PSUM) as ptr,
    ):
        scratch = scrp.tile([128, SPAN1], f)
        acc_tmp = scrp.tile([128, 1], f, tag="acctmp")
        for wstart in range(0, NOUT, SPAN1):
            width = min(SPAN1, NOUT - wstart)
            p_start = P0 + wstart
            ws = p_start - 259
            wwidth = width + 518
            xw = xw1p.tile([128, SPAN1 + 518], f)
            nc.gpsimd.dma_start(xw[0:C, 0:wwidth], x1[:, ws:ws + wwidth])
            nc.gpsimd.dma_start(xw[C:128, 0:wwidth], x2[:, ws:ws + wwidth])

            qkwin = qkwp.tile([128, SPAN1], f)
            for lc in range(0, width, 512):
                L = min(512, width - lc)
                # q and k accumulate in separate banks (own psum groups);
                # k writes partitions 64:128 so SBUF copies stay aligned.
                qps = pcv.tile([128, 512], f, tag="qps")
                kps = pcv.tile([128, 512], f, tag="kps")
                for t in range(9):
                    o = lc + TAP_OFF[t]
                    mm(qps[0:C, 0:L], w_lqkT[0:C, t * C:(t + 1) * C],
                       xw[0:C, o:o + L], start=(t == 0), stop=(t == 8),
                       tile_position=(0, 0))
                    mm(kps[C:128, 0:L], w_lqkT[C:128, t * C:(t + 1) * C],
                       xw[C:128, o:o + L], start=(t == 0), stop=(t == 8),
                       tile_position=(64, 64))
                nc.scalar.copy(qkwin[0:C, lc:lc + L], qps[0:C, 0:L])
                nc.scalar.copy(qkwin[C:128, lc:lc + L], kps[C:128, 0:L])

            # zero the per-row pad columns so they don't pollute S / norms
            if mode != "conv":
                for off, ln in _pad_positions(p_start, width):
                    nc.gpsimd.memset(qkwin[:, off:off + ln], 0.0)

            # norms: accumulate sum of squares over this window
            if mode not in ("conv", "convtr"):
                nc.scalar.activation(
                    scratch[:, 0:width], qkwin[:, 0:width],
                    mybir.ActivationFunctionType.Square, accum_out=acc_tmp[:])
                nc.vector.tensor_add(qk2[:], qk2[:], acc_tmp[:])

            # S += Tq.T @ Tk per 128-col sub-chunk
            if mode not in ("conv", "convttr"):
                for j in range(0, width, 128):
                    trps = ptr.tile([128, 128], f, tag="trps")
                    nc.tensor.transpose(trps[:], qkwin[:, j:j + 128], w_id[:])
                    trsb = trsbp.tile([128, 128], f)
                    nc.scalar.copy(trsb[:], trps[:])
                    mm(acc_ps[:], trsb[:, 0:C], trsb[:, C:128],
                       start=(sub_idx == 0), stop=(sub_idx == n_sub_total - 1))
                    sub_idx += 1

        if mode == "conv":
            # anchor so DCE can't drop the conv chain
            nc.sync.dma_start(y[:, 1, 0:C], qkwin[0:C, 0:C])

    # ---------------- collective: S and norms over the batch pair --------
    cc_sb = persist.tile([128, C + 1], f)
    nc.vector.memset(cc_sb[:], 0.0)
    if mode in ("conv", "convttr"):
        nc.scalar.copy(cc_sb[0:C, 0:C], w_pT[:])
    else:
        nc.scalar.copy(cc_sb[0:C, 0:C], acc_ps[:])
    nc.vector.tensor_copy(cc_sb[:, C:C + 1], qk2[:])
    if mode in ("conv", "convttr", "convtr"):
        nc.sync.dma_start(y[:, 0, 0:C + 1], cc_sb[0:C, :])
        for p in (persist, dram, wpool):
            p.release()
        accp.release()
        return
    cc_in = dram.tile([128, C + 1], f)
    cc_out = dram.tile([128, C + 1], f, tag="cc_out")
    nc.sync.dma_start(cc_in[:], cc_sb[:])
    if mode == "p1":
        nc.sync.dma_start(y[:, 0, 0:C + 1], cc_sb[0:C, :])
        for p in (persist, dram, wpool):
            p.release()
        accp.release()
        return
    if mode == "nocc":
        nc.gpsimd.dma_start(cc_out[:], cc_in[:])
    else:
        nc.gpsimd.collective_compute(
            "AllReduce", mybir.AluOpType.add,
            replica_groups=[[0, 1], [2, 3], [4, 5], [6, 7]],
            ins=[cc_in.opt()], outs=[cc_out.opt()],
        )
    sqk = persist.tile([128, C + 1], f, tag="sqk")
    nc.sync.dma_start(sqk[:], cc_out[:])
    if mode == "p1cc":
        nc.sync.dma_start(y[:, 0, 0:C + 1], sqk[0:C, :])
        for p in (persist, dram, wpool):
            p.release()
        accp.release()
        return

    # ---------------- tiny mid-section: softmax, M^T, G^T ----------------
    with (
        tc.tile_pool(name="mid", bufs=1) as midp,
        tc.tile_pool(name="ps_mid", bufs=1, space=bass.MemorySpace.PSUM) as pmid,
    ):
        nrm = midp.tile([128, 1], f, tag="nrm")       # sqrt of sums
        nc.scalar.sqrt(nrm[:], sqk[:, C:C + 1])
        nc.vector.tensor_scalar_max(nrm[:], nrm[:], EPS)
        rn = midp.tile([128, 1], f, tag="rn")         # 1/||.||
        nc.vector.reciprocal(rn[:], nrm[:])
        rs = midp.tile([C, 1], f, tag="rs")           # temp/||q|| per row c
        nc.vector.tensor_mul(rs[:], rn[0:C, :], w_temp[:])

        # broadcast 1/||k|| along free dim: transpose then rank-1 outer
        nkT_ps = pmid.tile([1, C], f, tag="nkT")
        nc.tensor.transpose(nkT_ps[:], rn[C:128, :], w_id[C:128, C:128])
        nkT = midp.tile([1, C], f, tag="nkT_sb")
        nc.scalar.copy(nkT[:], nkT_ps[:])
        ones1 = midp.tile([1, C], f, tag="ones1")
        nc.vector.memset(ones1[:], 1.0)
        nkb_ps = pmid.tile([C, C], f, tag="nkb")
        mm(nkb_ps[:], ones1[:], nkT[:])
        # logits = S * rs(row) * (1/||k||)(col)
        sp = midp.tile([C, C], f, tag="sp")
        nc.vector.tensor_scalar(sp[:], sqk[0:C, 0:C], rs[:], None,
                                op0=mybir.AluOpType.mult)
        nc.vector.tensor_mul(sp[:], sp[:], nkb_ps[:])

        # blockwise softmax via additive off-block mask (-1e30):
        # off-block entries exp to exactly 0, so the result IS Ablk.
        nc.vector.tensor_add(sp[:], sp[:], w_mask[:])
        negm = midp.tile([C, 1], f, tag="negm")
        nc.vector.tensor_reduce(negm[:], sp[:], axis=mybir.AxisListType.X,
                                op=mybir.AluOpType.max, negate=True)
        den = midp.tile([C, 1], f, tag="den")
        ex = midp.tile([C, C], f, tag="ex")
        nc.scalar.activation(ex[:], sp[:], mybir.ActivationFunctionType.Exp,
                             bias=negm[:], scale=1.0, accum_out=den[:])
        rden = midp.tile([C, 1], f, tag="rden")
        nc.vector.reciprocal(rden[:], den[:])
        ablk = midp.tile([C, C], f, tag="ablk")
        nc.vector.tensor_scalar(ablk[:], ex[:], rden[:], None,
                                op0=mybir.AluOpType.mult)

        # M^T = Ablk^T @ P^T
        mt_ps = pmid.tile([C, C], f, tag="mt")
        mm(mt_ps[:], ablk[:], w_pT[:])
        mt = midp.tile([C, C], f, tag="mt_sb")
        nc.scalar.copy(mt[:], mt_ps[:])

        # G^T[dp] = wv9[dp] @ M^T, duplicated to partitions 64:128
        gt_ps = pmid.tile([128, 9 * C], f, tag="gt")
        for dp in range(9):
            s = dp * C
            mm(gt_ps[0:C, s:s + C], w_wv9[:, s:s + C], mt[:],
               tile_position=(0, 0))
            mm(gt_ps[C:128, s:s + C], w_wv9[:, s:s + C], mt[:],
               tile_position=(0, 64))
        gt = persist.tile([128, 9 * C], f, tag="gt_sb")
        nc.scalar.copy(gt[:], gt_ps[:])

    accp.release()

    # ---------------- pass 2: out = G-conv(x2), write y ------------------
    with (
        tc.tile_pool(name="xw2", bufs=2) as xw2p,
        tc.tile_pool(name="osb", bufs=3) as osbp,
        tc.tile_pool(name="ps_p2", bufs=2, space=bass.MemorySpace.PSUM) as pp2,
    ):
        for rr in range(0, HH, ROWS2):
            nrows = min(ROWS2, HH - rr)
            p_start = P0 + rr * R
            ws = p_start - 259
            wwidth = nrows * R + 518
            xw = xw2p.tile([128, ROWS2 * R + 518], f)
            nc.gpsimd.dma_start(xw[0:C, 0:wwidth], x2[:, ws:ws + wwidth])
            nc.gpsimd.dma_start(xw[C:128, 0:wwidth], x2[:, ws:ws + wwidth])
            for r in range(nrows):
                base = r * R
                o2a = pp2.tile([C, R], f, tag="o2a")
                o2b = pp2.tile([C, R], f, tag="o2b")
                for t in range(9):
                    o = base + TAP_OFF[t]
                    if t % 2 == 0:
                        mm(o2a[:], gt[0:C, t * C:(t + 1) * C],
                           xw[0:C, o:o + R], start=(t == 0), stop=(t == 8),
                           tile_position=(0, 0))
                    else:
                        mm(o2b[:], gt[C:128, t * C:(t + 1) * C],
                           xw[C:128, o:o + R], start=(t == 1), stop=(t == 7),
                           tile_position=(64, 0))
                obs = osbp.tile([C, R], f, tag="obs")
                nc.scalar.copy(obs[:], o2b[:])
                osb = osbp.tile([C, R], f)
                nc.vector.tensor_add(osb[:], o2a[:], obs[:])
                nc.sync.dma_start(y[:, rr + r, :], osb[:, 1:W + 1])

    for p in (persist, dram, wpool):
        p.release()


# ======================= host side =========================================

def _prep_consts(q_w, q_dw_w, kv_w, kv_dw_w, proj_w, temperature):
    q_w = np.asarray(q_w, np.float32)[:, :, 0, 0]          # [o, i]
    kv_w = np.asarray(kv_w, np.float32)[:, :, 0, 0]        # [2C, i]
    q_dw = np.asarray(q_dw_w, np.float32)[:, 0]            # [C, 3, 3]
    kv_dw = np.asarray(kv_dw_w, np.float32)[:, 0]          # [2C, 3, 3]
    proj = np.asarray(proj_w, np.float32)[:, :, 0, 0]      # [o, c]
    temp = np.asarray(temperature, np.float32).reshape(HEADS)

    lqkT = np.zeros((128, 9 * C), np.float32)
    wv9 = np.zeros((C, 9 * C), np.float32)
    for t, (dy, dx) in enumerate(TAPS):
        w9q = q_dw[:, dy + 1, dx + 1][:, None] * q_w       # [o, i]
        w9k = kv_dw[0:C, dy + 1, dx + 1][:, None] * kv_w[0:C]
        lqkT[0:C, t * C:(t + 1) * C] = w9q.T
        lqkT[C:128, t * C:(t + 1) * C] = w9k.T
        # wv9[dp][d, i] = wdv[d, dp] * Wv[d, i]
        wv9[:, t * C:(t + 1) * C] = (
            kv_dw[C:2 * C, dy + 1, dx + 1][:, None] * kv_w[C:2 * C]
        )
    pTm = proj.T.copy()                                    # [c, o]
    temp64 = np.repeat(temp, HD).reshape(C, 1).astype(np.float32)
    ident = np.eye(128, dtype=np.float32)
    mask = np.full((C, C), -1e30, np.float32)
    for h in range(HEADS):
        mask[h * HD:(h + 1) * HD, h * HD:(h + 1) * HD] = 0.0
    return lqkT, wv9, pTm, temp64, ident, mask


def _prep_slice(img):
    """[C, H, W] -> padded flat [C, NBUF] per half; returns (top, bottom)."""
    out = []
    for h in range(2):
        buf = np.zeros((C, NR, R), np.float32)
        r0 = h * HH
        lo, hi = r0 - 1, r0 + HH + 1
        vlo, vhi = max(lo, 0), min(hi, H)
        buf[:, 1 + (vlo - lo):1 + (vlo - lo) + (vhi - vlo), 1:W + 1] = \
            img[:, vlo:vhi, :]
        out.append(np.ascontiguousarray(buf.reshape(C, NBUF)))
    return out


def kernel(input1, input2, q_w, q_dw_w, kv_w, kv_dw_w, proj_w, temperature):
    if "nc" not in _CACHE:
        _CACHE["nc"] = build_module()
    nc = _CACHE["nc"]

    lqkT, wv9, pTm, temp64, ident, mask = _prep_consts(
        q_w, q_dw_w, kv_w, kv_dw_w, proj_w, temperature)

    input1 = np.asarray(input1, np.float32)
    input2 = np.asarray(input2, np.float32)
    in_maps = []
    for core in range(8):
        b, h = core // 2, core % 2
        x1t = _prep_slice(input1[b])[h]
        x2t = _prep_slice(input2[b])[h]
        in_maps.append({
            "x1": x1t, "x2": x2t, "lqkT": lqkT, "wv9": wv9, "pT": pTm,
            "temp64": temp64, "ident": ident, "mask64": mask,
        })

    results = _get_runner(nc)(in_maps)
    out = np.empty((B, C, H, W), np.float32)
    for core in range(8):
        b, h = core // 2, core % 2
        out[b, :, h * HH:(h + 1) * HH, :] = results[core]["y"]
    return out


def _get_runner(nc, n_cores=8):
    """Like bass2jax.run_bass_via_pjrt, but the jitted shard_map executable is
    built once and reused across calls (avoids per-call retracing)."""
    if "runner" in _CACHE:
        return _CACHE["runner"]
    import jax
    from jax.sharding import Mesh, PartitionSpec
    from jax.experimental.shard_map import shard_map
    from concourse import bass2jax as b2j
    from concourse import mybir as _mb

    b2j.install_neuronx_cc_hook()
    partition_name = nc.partition_id_tensor.name if nc.partition_id_tensor else None
    in_names, out_names, out_avals, zero_shapes = [], [], [], []
    for alloc in nc.m.functions[0].allocations:
        if not isinstance(alloc, _mb.MemoryLocationSet):
            continue
        name = alloc.memorylocations[0].name
        if alloc.kind == "ExternalInput":
            if name != partition_name:
                in_names.append(name)
        elif alloc.kind == "ExternalOutput":
            out_names.append(name)
            shape = tuple(alloc.tensor_shape)
            dtype = _mb.dt.np(alloc.dtype)
            out_avals.append(jax.core.ShapedArray(shape, dtype))
            zero_shapes.append((shape, dtype))
    n_params = len(in_names)
    n_outs = len(out_avals)
    all_in_names = list(in_names) + list(out_names)
    if partition_name is not None:
        all_in_names.append(partition_name)
    donate = tuple(range(n_params, n_params + n_outs))

    def _pjrt_body(*args):
        operands = list(args)
        if partition_name is not None:
            operands.append(b2j.partition_id_tensor())
        return tuple(b2j._bass_exec_p.bind(
            *operands, out_avals=tuple(out_avals), in_names=tuple(all_in_names),
            out_names=tuple(out_names), lowering_input_output_aliases=(),
            sim_require_finite=True, sim_require_nnan=True, nc=nc))

    devices = jax.devices()[:n_cores]
    mesh = Mesh(np.asarray(devices), ("core",))
    sharded = jax.jit(
        shard_map(_pjrt_body, mesh=mesh,
                  in_specs=(PartitionSpec("core"),) * (n_params + n_outs),
                  out_specs=(PartitionSpec("core"),) * n_outs, check_rep=False),
        donate_argnums=donate, keep_unused=True)

    def run(in_maps):
        concat_in = [
            np.concatenate([np.asarray(in_maps[c][nm]) for c in range(n_cores)], 0)
            for nm in in_names
        ]
        concat_zeros = [np.zeros((n_cores * s[0], *s[1:]), d)
                        for s, d in zero_shapes]
        out_arrs = sharded(*concat_in, *concat_zeros)
        return [
            {nm: np.asarray(out_arrs[i]).reshape(n_cores, *out_avals[i].shape)[c]
             for i, nm in enumerate(out_names)}
            for c in range(n_cores)
        ]

    _CACHE["runner"] = run
    return run



# revision 4
# speedup vs baseline: 18245.2935x; 18245.2935x over previous
"""CAB (channel-attention block) Trainium2 kernel, bf16 datapath.

Sharding: 8 cores = 4 batches x 2 H-halves. Each core computes its
[64, 128, 256] slice of the output. The q.kT contraction and the L2-norm
sums are AllReduced across the 2 cores sharing a batch (sequence-parallel).

Math folds (all exact):
  - L2 normalize folds into S: attn_logits = S * temp / (||q|| ||k||^T),
    with S = q @ k^T computed on raw (unnormalized) q, k.
  - proj o (attn @ v) o dwconv_v o pwconv_v folds into a single 3x3 dense
    conv on input2 with data-dependent matrices
    G[dp] = (P @ A_blockdiag) @ (diag(wdv[:,dp]) @ Wv).

PE packing:
  - pass 1: q-conv (x1) and k-conv (x2) run as ONE matmul per tap with a
    block-diagonal [128,128] lhsT and rhs = [x1; x2] stacked on partitions
    (K=128, M=128) -> 9 PE cycles/col for both convs.
  - pass 2: rhs = [x2 @ off; x2 @ off+R] (row-shifted copies on the two
    partition halves); each [128,128] lhsT computes TWO output rows at
    once -> 6 matmuls per row-pair (3 PE cycles/col).
"""
import sys

sys.path.insert(0, "/opt/trn_rl_repo")

import numpy as np
import ml_dtypes

import concourse.bacc as bacc
import concourse.bass as bass
import concourse.tile as tile
from concourse import mybir
from concourse.bass_utils import run_bass_kernel_spmd

F32 = mybir.dt.float32
BF16 = mybir.dt.bfloat16
NP_BF16 = ml_dtypes.bfloat16

B, C, H, W = 4, 64, 256, 256
HEADS = 8
HD = C // HEADS
EPS = 1e-12

HH = H // 2            # rows per core
R = W + 2              # padded row length
NR = HH + 4            # buffer rows: zero, halo, 128 data, halo, zero
NBUF = NR * R          # per-core padded input length (flattened)
P0 = 2 * R             # first output position (data row 0, col -1(pad))
NOUT = HH * R          # output span incl. per-row col pads

TAPS = [(dy, dx) for dy in (-1, 0, 1) for dx in (-1, 0, 1)]
# rhs offset of tap within a window that starts 259 cols before the chunk
TAP_OFF = [259 + dy * R + dx for dy, dx in TAPS]

SPAN1 = 4096           # pass-1 window span (multiple of 128)
CHUNK = 512            # pass-1 psum chunk (one bank)
PAIRS2 = 8             # pass-2 row-pairs per window

_CACHE = {}


def _pad_positions(start, length):
    """Contiguous runs of pad columns (global col % R in {0, R-1}) within
    [start, start+length), as (offset_rel, run_len) with runs <= 2."""
    runs = []
    end = start + length
    p = (start // R) * R - 1
    while p < end:
        for q in (p, p + 1):  # (row r col 257, row r+1 col 0) adjacent pair
            lo = max(q, start)
            hi = min(q + 1, end)
            if lo < hi:
                if runs and runs[-1][0] + runs[-1][1] == lo - start:
                    runs[-1] = (runs[-1][0], runs[-1][1] + (hi - lo))
                else:
                    runs.append((lo - start, hi - lo))
        p += R
    return runs


def build_module():
    nc = bacc.Bacc("TRN2", target_bir_lowering=False, debug=False, num_devices=8)

    x1 = nc.declare_dram_parameter("x1", [C, NBUF], BF16, isOutput=False)
    x2 = nc.declare_dram_parameter("x2", [C, NBUF], BF16, isOutput=False)
    lqk9 = nc.declare_dram_parameter("lqk9", [128, 9 * 128], BF16, isOutput=False)
    wv9 = nc.declare_dram_parameter("wv9", [C, 9 * C], F32, isOutput=False)
    pT = nc.declare_dram_parameter("pT", [C, C], F32, isOutput=False)
    temp64 = nc.declare_dram_parameter("temp64", [C, 1], F32, isOutput=False)
    ident = nc.declare_dram_parameter("ident", [128, 128], F32, isOutput=False)
    identb = nc.declare_dram_parameter("identb", [128, 128], BF16, isOutput=False)
    mask64 = nc.declare_dram_parameter("mask64", [C, C], F32, isOutput=False)
    y = nc.declare_dram_parameter("y", [C, HH, W], BF16, isOutput=True)

    with tile.TileContext(nc) as tc:
        _body(tc, nc, x1, x2, lqk9, wv9, pT, temp64, ident, identb, mask64, y)
    nc.compile()
    return nc


def _body(tc, nc, x1, x2, lqk9, wv9, pT, temp64, ident, identb, mask64, y):
    mm = nc.tensor.matmul
    f = F32

    wpool = tc.alloc_tile_pool(name="weights", bufs=1)
    dram = tc.alloc_tile_pool(name="dram", bufs=1, space="DRAM")
    accp = tc.alloc_tile_pool(name="ps_acc", bufs=1, space=bass.MemorySpace.PSUM)
    persist = tc.alloc_tile_pool(name="persist", bufs=1)

    w_lqk9 = wpool.tile([128, 9 * 128], BF16)
    nc.gpsimd.dma_start(w_lqk9[:], lqk9[:])
    w_wv9 = wpool.tile([C, 9 * C], f)
    nc.gpsimd.dma_start(w_wv9[:], wv9[:])
    w_pT = wpool.tile([C, C], f)
    nc.gpsimd.dma_start(w_pT[:], pT[:])
    w_temp = wpool.tile([C, 1], f)
    nc.gpsimd.dma_start(w_temp[:], temp64[:])
    w_id = wpool.tile([128, 128], f)
    nc.gpsimd.dma_start(w_id[:], ident[:])
    w_idb = wpool.tile([128, 128], BF16)
    nc.gpsimd.dma_start(w_idb[:], identb[:])
    w_mask = wpool.tile([C, C], f)
    nc.gpsimd.dma_start(w_mask[:], mask64[:])

    acc_ps = accp.tile([C, C], f)          # S accumulator (q.kT)
    qk2 = persist.tile([128, 1], f)        # running sum q^2 (top) / k^2 (bottom)
    nc.vector.memset(qk2[:], 0.0)

    # pass-2 window prefetch machinery (tiles loaded before the collective
    # so the DMA overlaps the AllReduce latency)
    n_pairs = HH // 2
    n_win2 = (n_pairs + PAIRS2 - 1) // PAIRS2
    xw2p = tc.alloc_tile_pool(name="xw2", bufs=4)
    xw2_tiles = {}

    def load_win2(wi):
        base_pair = wi * PAIRS2
        np_w = min(PAIRS2, n_pairs - base_pair)
        rr = 2 * base_pair
        start0 = P0 + (rr - 1) * R - 1
        wwidth = (2 * np_w + 1) * R + 2
        xw = xw2p.tile([128, (2 * PAIRS2 + 1) * R + 2], BF16)
        nc.gpsimd.dma_start(xw[0:C, 0:wwidth], x2[:, start0:start0 + wwidth])
        nc.gpsimd.dma_start(xw[C:128, 0:wwidth],
                            x2[:, start0 + R:start0 + R + wwidth])
        xw2_tiles[wi] = xw

    # ---------------- pass 1: q,k conv -> transpose -> S, norms ----------
    n_blk_total = NOUT // 128
    windows = []
    ws_ = 0
    while ws_ < NOUT:
        windows.append((ws_, min(SPAN1, NOUT - ws_)))
        ws_ += SPAN1

    with (
        tc.tile_pool(name="xw1", bufs=2) as xw1p,
        tc.tile_pool(name="qkwin", bufs=2) as qkwp,
        tc.tile_pool(name="trsb", bufs=3) as trsbp,
        tc.tile_pool(name="scratch", bufs=1) as scrp,
        tc.tile_pool(name="acct", bufs=2) as acctp,
        tc.tile_pool(name="ps_conv", bufs=2, space=bass.MemorySpace.PSUM) as pcv,
        tc.tile_pool(name="ps_tr", bufs=2, space=bass.MemorySpace.PSUM) as ptr,
    ):
        scratch = scrp.tile([128, SPAN1], BF16)
        blk_idx = 0
        prev = None    # (qkwin_tile, width) pending transposes

        def do_transposes(qkwin, width):
            nonlocal blk_idx
            for j in range(0, width, 128):
                trps = ptr.tile([128, 128], BF16, tag="trps")
                nc.tensor.transpose(trps[:], qkwin[:, j:j + 128], w_idb[:])
                trsb = trsbp.tile([128, 128], BF16)
                nc.vector.tensor_copy(trsb[:], trps[:])
                mm(acc_ps[:], trsb[:, 0:C], trsb[:, C:128],
                   start=(blk_idx == 0), stop=(blk_idx == n_blk_total - 1))
                blk_idx += 1

        for wstart, width in windows:
            p_start = P0 + wstart
            ws = p_start - 259
            wwidth = width + 518
            xw = xw1p.tile([128, SPAN1 + 518], BF16)
            nc.gpsimd.dma_start(xw[0:C, 0:wwidth], x1[:, ws:ws + wwidth])
            nc.gpsimd.dma_start(xw[C:128, 0:wwidth], x2[:, ws:ws + wwidth])

            qkwin = qkwp.tile([128, SPAN1], BF16)
            for lc in range(0, width, CHUNK):
                L = min(CHUNK, width - lc)
                qk_ps = pcv.tile([128, CHUNK], f, tag="qkps")
                for t in range(9):
                    o = lc + TAP_OFF[t]
                    mm(qk_ps[:, 0:L], w_lqk9[:, t * 128:(t + 1) * 128],
                       xw[:, o:o + L], start=(t == 0), stop=(t == 8))
                nc.vector.tensor_copy(qkwin[:, lc:lc + L], qk_ps[:, 0:L])

            # zero the per-row pad columns so they don't pollute S / norms
            for off, ln in _pad_positions(p_start, width):
                nc.gpsimd.memset(qkwin[:, off:off + ln], 0.0)

            # norms: accumulate sum of squares over this window
            acc_tmp = acctp.tile([128, 1], f)
            nc.scalar.activation(
                scratch[:, 0:width], qkwin[:, 0:width],
                mybir.ActivationFunctionType.Square, accum_out=acc_tmp[:])
            nc.vector.tensor_add(qk2[:], qk2[:], acc_tmp[:])

            # software pipeline: transposes of the PREVIOUS window run after
            # this window's conv matmuls, keeping PE fed while copies drain
            if prev is not None:
                do_transposes(*prev)
            prev = (qkwin, width)

        do_transposes(*prev)

    # prefetch first pass-2 windows; DMA overlaps the collective below
    for wi in range(min(3, n_win2)):
        load_win2(wi)

    # ---------------- collective: S and norms over the batch pair --------
    cc_sb = persist.tile([128, C + 1], f)
    nc.vector.memset(cc_sb[:], 0.0)
    nc.scalar.copy(cc_sb[0:C, 0:C], acc_ps[:])
    nc.vector.tensor_copy(cc_sb[:, C:C + 1], qk2[:])
    cc_in = dram.tile([128, C + 1], f)
    cc_out = dram.tile([128, C + 1], f, tag="cc_out")
    nc.sync.dma_start(cc_in[:], cc_sb[:])
    nc.gpsimd.collective_compute(
        "AllReduce", mybir.AluOpType.add,
        replica_groups=[[0, 1], [2, 3], [4, 5], [6, 7]],
        ins=[cc_in.opt()], outs=[cc_out.opt()],
    )
    sqk = persist.tile([128, C + 1], f, tag="sqk")
    nc.sync.dma_start(sqk[:], cc_out[:])

    # ---------------- tiny mid-section: softmax, M^T, G^T, pass-2 lhsT ---
    g6 = []   # 6 bf16 [128,128] lhsT tiles for pass 2: (parity, dx)
    with (
        tc.tile_pool(name="mid", bufs=1) as midp,
        tc.tile_pool(name="ps_mid", bufs=1, space=bass.MemorySpace.PSUM) as pmid,
    ):
        nrm = midp.tile([128, 1], f, tag="nrm")       # sqrt of sums
        nc.scalar.sqrt(nrm[:], sqk[:, C:C + 1])
        nc.vector.tensor_scalar_max(nrm[:], nrm[:], EPS)
        rn = midp.tile([128, 1], f, tag="rn")         # 1/||.||
        nc.vector.reciprocal(rn[:], nrm[:])
        rs = midp.tile([C, 1], f, tag="rs")           # temp/||q|| per row c
        nc.vector.tensor_mul(rs[:], rn[0:C, :], w_temp[:])

        # broadcast 1/||k|| along free dim: transpose then rank-1 outer
        nkT_ps = pmid.tile([1, C], f, tag="nkT")
        nc.tensor.transpose(nkT_ps[:], rn[C:128, :], w_id[C:128, C:128])
        nkT = midp.tile([1, C], f, tag="nkT_sb")
        nc.scalar.copy(nkT[:], nkT_ps[:])
        ones1 = midp.tile([1, C], f, tag="ones1")
        nc.vector.memset(ones1[:], 1.0)
        nkb_ps = pmid.tile([C, C], f, tag="nkb")
        mm(nkb_ps[:], ones1[:], nkT[:])
        # logits = S * rs(row) * (1/||k||)(col)
        sp = midp.tile([C, C], f, tag="sp")
        nc.vector.tensor_scalar(sp[:], sqk[0:C, 0:C], rs[:], None,
                                op0=mybir.AluOpType.mult)
        nc.vector.tensor_mul(sp[:], sp[:], nkb_ps[:])

        # blockwise softmax via additive off-block mask (-1e30):
        # off-block entries exp to exactly 0, so the result IS Ablk.
        nc.vector.tensor_add(sp[:], sp[:], w_mask[:])
        negm = midp.tile([C, 1], f, tag="negm")
        nc.vector.tensor_reduce(negm[:], sp[:], axis=mybir.AxisListType.X,
                                op=mybir.AluOpType.max, negate=True)
        den = midp.tile([C, 1], f, tag="den")
        ex = midp.tile([C, C], f, tag="ex")
        nc.scalar.activation(ex[:], sp[:], mybir.ActivationFunctionType.Exp,
                             bias=negm[:], scale=1.0, accum_out=den[:])
        rden = midp.tile([C, 1], f, tag="rden")
        nc.vector.reciprocal(rden[:], den[:])
        ablk = midp.tile([C, C], f, tag="ablk")
        nc.vector.tensor_scalar(ablk[:], ex[:], rden[:], None,
                                op0=mybir.AluOpType.mult)

        # M^T = Ablk^T @ P^T
        mt_ps = pmid.tile([C, C], f, tag="mt")
        mm(mt_ps[:], ablk[:], w_pT[:])
        mt = midp.tile([C, C], f, tag="mt_sb")
        nc.scalar.copy(mt[:], mt_ps[:])

        # G^T[dp] = wv9[dp] @ M^T, computed into both partition halves so
        # the 6 pass-2 lhsT tiles can be assembled with lane-aligned copies
        gt_ps = pmid.tile([128, 9 * C], f, tag="gt")
        for dp in range(9):
            s = dp * C
            mm(gt_ps[0:C, s:s + C], w_wv9[:, s:s + C], mt[:],
               tile_position=(0, 0))
            mm(gt_ps[C:128, s:s + C], w_wv9[:, s:s + C], mt[:],
               tile_position=(0, 64))

        # pass-2 stationary tiles:
        #   A(dx) = [[G(-1,dx)^T, 0         ], [G(0,dx)^T,  G(-1,dx)^T]]
        #   B(dx) = [[G(+1,dx)^T, G(0,dx)^T ], [0,          G(+1,dx)^T]]
        # (quadrants as [k-range 0:64 | 64:128] x [m-range 0:64 | 64:128])
        def tidx(dy, dx):
            return ((dy + 1) * 3 + (dx + 1)) * C

        for dx in (-1, 0, 1):
            ga = persist.tile([128, 128], BF16, tag=f"gA{dx}")
            nc.vector.memset(ga[:], 0.0)
            sm1, s0 = tidx(-1, dx), tidx(0, dx)
            nc.scalar.copy(ga[0:C, 0:C], gt_ps[0:C, sm1:sm1 + C])
            nc.scalar.copy(ga[C:128, 0:C], gt_ps[C:128, s0:s0 + C])
            nc.scalar.copy(ga[C:128, C:128], gt_ps[C:128, sm1:sm1 + C])
            gb = persist.tile([128, 128], BF16, tag=f"gB{dx}")
            nc.vector.memset(gb[:], 0.0)
            sp1 = tidx(1, dx)
            nc.scalar.copy(gb[0:C, 0:C], gt_ps[0:C, sp1:sp1 + C])
            nc.scalar.copy(gb[0:C, C:128], gt_ps[0:C, s0:s0 + C])
            nc.scalar.copy(gb[C:128, C:128], gt_ps[C:128, sp1:sp1 + C])
            g6.append((ga, gb))

    accp.release()

    # ---------------- pass 2: out = G-conv(x2) via row pairs, write y ----
    with (
        tc.tile_pool(name="osb", bufs=4) as osbp,
        tc.tile_pool(name="ps_p2", bufs=4, space=bass.MemorySpace.PSUM) as pp2,
    ):
        for wi in range(n_win2):
            if wi + 3 < n_win2:
                load_win2(wi + 3)
            base_pair = wi * PAIRS2
            np_w = min(PAIRS2, n_pairs - base_pair)
            xw = xw2_tiles.pop(wi)
            for p in range(np_w):
                ps = pp2.tile([128, R], f, tag="o2")
                for i, dx in enumerate((-1, 0, 1)):
                    oA = (2 * p) * R + dx + 1
                    oB = (2 * p + 2) * R + dx + 1
                    ga, gb = g6[i]
                    mm(ps[:], ga[:], xw[:, oA:oA + R],
                       start=(i == 0), stop=False)
                    mm(ps[:], gb[:], xw[:, oB:oB + R],
                       start=False, stop=(i == 2))
                obs = osbp.tile([128, R], BF16)
                nc.vector.tensor_copy(obs[:], ps[:])
                row = 2 * (base_pair + p)
                nc.sync.dma_start(y[:, row, :], obs[0:C, 1:W + 1])
                nc.sync.dma_start(y[:, row + 1, :], obs[C:128, 1:W + 1])

    xw2p.release()
    for p in (persist, dram, wpool):
        p.release()


# ======================= host side =========================================

def _prep_consts(q_w, q_dw_w, kv_w, kv_dw_w, proj_w, temperature):
    q_w = np.asarray(q_w, np.float32)[:, :, 0, 0]          # [o, i]
    kv_w = np.asarray(kv_w, np.float32)[:, :, 0, 0]        # [2C, i]
    q_dw = np.asarray(q_dw_w, np.float32)[:, 0]            # [C, 3, 3]
    kv_dw = np.asarray(kv_dw_w, np.float32)[:, 0]          # [2C, 3, 3]
    proj = np.asarray(proj_w, np.float32)[:, :, 0, 0]      # [o, c]
    temp = np.asarray(temperature, np.float32).reshape(HEADS)

    lqk9 = np.zeros((128, 9 * 128), np.float32)
    wv9 = np.zeros((C, 9 * C), np.float32)
    for t, (dy, dx) in enumerate(TAPS):
        w9q = q_dw[:, dy + 1, dx + 1][:, None] * q_w       # [o, i]
        w9k = kv_dw[0:C, dy + 1, dx + 1][:, None] * kv_w[0:C]
        lqk9[0:C, t * 128:t * 128 + C] = w9q.T
        lqk9[C:128, t * 128 + C:(t + 1) * 128] = w9k.T
        # wv9[dp][d, i] = wdv[d, dp] * Wv[d, i]
        wv9[:, t * C:(t + 1) * C] = (
            kv_dw[C:2 * C, dy + 1, dx + 1][:, None] * kv_w[C:2 * C]
        )
    pTm = proj.T.copy()                                    # [c, o]
    temp64 = np.repeat(temp, HD).reshape(C, 1).astype(np.float32)
    ident = np.eye(128, dtype=np.float32)
    identb = np.eye(128, dtype=NP_BF16)
    mask = np.full((C, C), -1e30, np.float32)
    for h in range(HEADS):
        mask[h * HD:(h + 1) * HD, h * HD:(h + 1) * HD] = 0.0
    return lqk9.astype(NP_BF16), wv9, pTm, temp64, ident, identb, mask


def _prep_slices(img_bf):
    """[C, H, W] bf16 -> padded flat [C, NBUF] per half; returns (top, bot)."""
    out = []
    for h in range(2):
        buf = np.zeros((C, NR, R), NP_BF16)
        r0 = h * HH
        lo, hi = r0 - 1, r0 + HH + 1
        vlo, vhi = max(lo, 0), min(hi, H)
        buf[:, 1 + (vlo - lo):1 + (vlo - lo) + (vhi - vlo), 1:W + 1] = \
            img_bf[:, vlo:vhi, :]
        out.append(np.ascontiguousarray(buf.reshape(C, NBUF)))
    return out


def _build_in_maps(input1, input2, q_w, q_dw_w, kv_w, kv_dw_w, proj_w,
                   temperature):
    lqk9, wv9, pTm, temp64, ident, identb, mask = _prep_consts(
        q_w, q_dw_w, kv_w, kv_dw_w, proj_w, temperature)
    in1_bf = np.asarray(input1, np.float32).astype(NP_BF16)
    in2_bf = np.asarray(input2, np.float32).astype(NP_BF16)
    in_maps = []
    for core in range(8):
        b, h = core // 2, core % 2
        x1t = _prep_slices(in1_bf[b])[h]
        x2t = _prep_slices(in2_bf[b])[h]
        in_maps.append({
            "x1": x1t, "x2": x2t, "lqk9": lqk9, "wv9": wv9, "pT": pTm,
            "temp64": temp64, "ident": ident, "identb": identb,
            "mask64": mask,
        })
    return in_maps


def kernel(input1, input2, q_w, q_dw_w, kv_w, kv_dw_w, proj_w, temperature):
    if "nc" not in _CACHE:
        _CACHE["nc"] = build_module()
    nc = _CACHE["nc"]

    in_maps = _build_in_maps(input1, input2, q_w, q_dw_w, kv_w, kv_dw_w,
                             proj_w, temperature)
    results = _get_runner(nc)(in_maps)
    out = np.empty((B, C, H, W), np.float32)
    for core in range(8):
        b, h = core // 2, core % 2
        out[b, :, h * HH:(h + 1) * HH, :] = \
            results[core]["y"].astype(np.float32)
    return out


def _get_runner(nc, n_cores=8):
    """Like bass2jax.run_bass_via_pjrt, but the jitted shard_map executable is
    built once and reused across calls (avoids per-call retracing)."""
    if "runner" in _CACHE:
        return _CACHE["runner"]
    import jax
    from jax.sharding import Mesh, PartitionSpec
    from jax.experimental.shard_map import shard_map
    from concourse import bass2jax as b2j
    from concourse import mybir as _mb

    b2j.install_neuronx_cc_hook()
    partition_name = nc.partition_id_tensor.name if nc.partition_id_tensor else None
    in_names, out_names, out_avals, zero_shapes = [], [], [], []
    for alloc in nc.m.functions[0].allocations:
        if not isinstance(alloc, _mb.MemoryLocationSet):
            continue
        name = alloc.memorylocations[0].name
        if alloc.kind == "ExternalInput":
            if name != partition_name:
                in_names.append(name)
        elif alloc.kind == "ExternalOutput":
            out_names.append(name)
            shape = tuple(alloc.tensor_shape)
            dtype = _mb.dt.np(alloc.dtype)
            out_avals.append(jax.core.ShapedArray(shape, dtype))
            zero_shapes.append((shape, dtype))
    n_params = len(in_names)
    n_outs = len(out_avals)
    all_in_names = list(in_names) + list(out_names)
    if partition_name is not None:
        all_in_names.append(partition_name)
    donate = tuple(range(n_params, n_params + n_outs))

    def _pjrt_body(*args):
        operands = list(args)
        if partition_name is not None:
            operands.append(b2j.partition_id_tensor())
        return tuple(b2j._bass_exec_p.bind(
            *operands, out_avals=tuple(out_avals), in_names=tuple(all_in_names),
            out_names=tuple(out_names), lowering_input_output_aliases=(),
            sim_require_finite=True, sim_require_nnan=True, nc=nc))

    devices = jax.devices()[:n_cores]
    mesh = Mesh(np.asarray(devices), ("core",))
    sharded = jax.jit(
        shard_map(_pjrt_body, mesh=mesh,
                  in_specs=(PartitionSpec("core"),) * (n_params + n_outs),
                  out_specs=(PartitionSpec("core"),) * n_outs, check_rep=False),
        donate_argnums=donate, keep_unused=True)

    def run(in_maps):
        concat_in = [
            np.concatenate([np.asarray(in_maps[c][nm]) for c in range(n_cores)], 0)
            for nm in in_names
        ]
        concat_zeros = [np.zeros((n_cores * s[0], *s[1:]), d)
                        for s, d in zero_shapes]
        out_arrs = sharded(*concat_in, *concat_zeros)
        return [
            {nm: np.asarray(out_arrs[i]).reshape(n_cores, *out_avals[i].shape)[c]
             for i, nm in enumerate(out_names)}
            for c in range(n_cores)
        ]

    _CACHE["runner"] = run
    return run


# revision 12
# speedup vs baseline: 19126.9584x; 1.0483x over previous
"""CAB (channel-attention block) Trainium2 kernel, bf16 datapath.

Sharding: 8 cores = 4 batches x 2 H-halves. Each core computes its
[64, 128, 256] slice of the output. The q.kT contraction and the L2-norm
sums are AllReduced across the 2 cores sharing a batch (sequence-parallel).

Math folds (all exact):
  - L2 normalize folds into S: attn_logits = S * temp / (||q|| ||k||^T),
    with S = q @ k^T computed on raw (unnormalized) q, k.
  - proj o (attn @ v) o dwconv_v o pwconv_v folds into a single 3x3 dense
    conv on input2 with data-dependent matrices
    G[dp] = (P @ A_blockdiag) @ (diag(wdv[:,dp]) @ Wv).

PE packing:
  - pass 1: q-conv (x1) and k-conv (x2) run as ONE matmul per tap with a
    block-diagonal [128,128] lhsT and rhs = [x1; x2] stacked on partitions
    (K=128, M=128) -> 9 PE cycles/col for both convs.
  - pass 2: rhs = [x2 @ off; x2 @ off+R] (row-shifted copies on the two
    partition halves); each [128,128] lhsT computes TWO output rows at
    once -> 6 matmuls per row-pair (3 PE cycles/col).
"""
import sys

sys.path.insert(0, "/opt/trn_rl_repo")

import numpy as np
import ml_dtypes

import concourse.bacc as bacc
import concourse.bass as bass
import concourse.tile as tile
from concourse import mybir
from concourse.bass_utils import run_bass_kernel_spmd

F32 = mybir.dt.float32
BF16 = mybir.dt.bfloat16
NP_BF16 = ml_dtypes.bfloat16

B, C, H, W = 4, 64, 256, 256
HEADS = 8
HD = C // HEADS
EPS = 1e-12

HH = H // 2            # rows per core
R = W + 2              # padded row length
NR = HH + 4            # buffer rows: zero, halo, 128 data, halo, zero
NBUF = NR * R          # per-core padded input length (flattened)
P0 = 2 * R             # first output position (data row 0, col -1(pad))
NOUT = HH * R          # output span incl. per-row col pads

TAPS = [(dy, dx) for dy in (-1, 0, 1) for dx in (-1, 0, 1)]
# rhs offset of tap within a window that starts 259 cols before the chunk
TAP_OFF = [259 + dy * R + dx for dy, dx in TAPS]

SPAN1 = 4096           # pass-1 window span (multiple of 128)
CHUNK = 512            # pass-1 psum chunk (one bank)
PAIRS2 = 8             # pass-2 row-pairs per window

_CACHE = {}


def _pad_positions(start, length):
    """Contiguous runs of pad columns (global col % R in {0, R-1}) within
    [start, start+length), as (offset_rel, run_len) with runs <= 2."""
    runs = []
    end = start + length
    p = (start // R) * R - 1
    while p < end:
        for q in (p, p + 1):  # (row r col 257, row r+1 col 0) adjacent pair
            lo = max(q, start)
            hi = min(q + 1, end)
            if lo < hi:
                if runs and runs[-1][0] + runs[-1][1] == lo - start:
                    runs[-1] = (runs[-1][0], runs[-1][1] + (hi - lo))
                else:
                    runs.append((lo - start, hi - lo))
        p += R
    return runs


def build_module():
    nc = bacc.Bacc("TRN2", target_bir_lowering=False, debug=False, num_devices=8)

    x1 = nc.declare_dram_parameter("x1", [C, NBUF], BF16, isOutput=False)
    x2 = nc.declare_dram_parameter("x2", [C, NBUF], BF16, isOutput=False)
    lqk9 = nc.declare_dram_parameter("lqk9", [128, 9 * 128], BF16, isOutput=False)
    wv9 = nc.declare_dram_parameter("wv9", [C, 9 * C], F32, isOutput=False)
    pT = nc.declare_dram_parameter("pT", [C, C], F32, isOutput=False)
    temp64 = nc.declare_dram_parameter("temp64", [C, 1], F32, isOutput=False)
    ident = nc.declare_dram_parameter("ident", [128, 128], F32, isOutput=False)
    identb = nc.declare_dram_parameter("identb", [128, 128], BF16, isOutput=False)
    mask64 = nc.declare_dram_parameter("mask64", [C, C], F32, isOutput=False)
    y = nc.declare_dram_parameter("y", [C, HH, W], BF16, isOutput=True)

    with tile.TileContext(nc) as tc:
        _body(tc, nc, x1, x2, lqk9, wv9, pT, temp64, ident, identb, mask64, y)
    nc.compile()
    return nc


def _body(tc, nc, x1, x2, lqk9, wv9, pT, temp64, ident, identb, mask64, y):
    mm = nc.tensor.matmul
    f = F32

    wpool = tc.alloc_tile_pool(name="weights", bufs=1)
    dram = tc.alloc_tile_pool(name="dram", bufs=1, space="DRAM")
    accp = tc.alloc_tile_pool(name="ps_acc", bufs=1, space=bass.MemorySpace.PSUM)
    persist = tc.alloc_tile_pool(name="persist", bufs=1)

    # only the pass-1 weights load up-front; mid-section weights load later
    # so the first x-window DMA isn't queued behind them
    w_lqk9 = wpool.tile([128, 9 * 128], BF16)
    nc.gpsimd.dma_start(w_lqk9[:], lqk9[:])
    w_idb = wpool.tile([128, 128], BF16)
    nc.gpsimd.dma_start(w_idb[:], identb[:])
    w_wv9 = wpool.tile([C, 9 * C], f)
    w_pT = wpool.tile([C, C], f)
    w_temp = wpool.tile([C, 1], f)
    w_id = wpool.tile([128, 128], f)
    w_mask = wpool.tile([C, C], f)

    acc_ps = accp.tile([C, C], f)          # S accumulator (q.kT)
    qk2 = persist.tile([128, 1], f)        # running sum q^2 (top) / k^2 (bottom)
    nc.vector.memset(qk2[:], 0.0)

    # pass-2 window prefetch machinery (tiles loaded before the collective
    # so the DMA overlaps the AllReduce latency)
    n_pairs = HH // 2
    n_win2 = (n_pairs + PAIRS2 - 1) // PAIRS2
    xw2p = tc.alloc_tile_pool(name="xw2", bufs=n_win2)
    xw2_tiles = {}

    def load_win2(wi):
        base_pair = wi * PAIRS2
        np_w = min(PAIRS2, n_pairs - base_pair)
        rr = 2 * base_pair
        start0 = P0 + (rr - 1) * R - 1
        wwidth = (2 * np_w + 1) * R + 2
        xw = xw2p.tile([128, (2 * PAIRS2 + 1) * R + 2], BF16)
        nc.gpsimd.dma_start(xw[0:C, 0:wwidth], x2[:, start0:start0 + wwidth])
        nc.gpsimd.dma_start(xw[C:128, 0:wwidth],
                            x2[:, start0 + R:start0 + R + wwidth])
        xw2_tiles[wi] = xw

    # ---------------- pass 1: q,k conv -> transpose -> S, norms ----------
    n_blk_total = NOUT // 128
    windows = []
    ws_ = 0
    while ws_ < NOUT:
        windows.append((ws_, min(SPAN1, NOUT - ws_)))
        ws_ += SPAN1

    with (
        tc.tile_pool(name="xw1", bufs=2) as xw1p,
        tc.tile_pool(name="qkwin", bufs=2) as qkwp,
        tc.tile_pool(name="trsb", bufs=3) as trsbp,
        tc.tile_pool(name="scratch", bufs=1) as scrp,
        tc.tile_pool(name="acct", bufs=2) as acctp,
        tc.tile_pool(name="ps_conv", bufs=4, space=bass.MemorySpace.PSUM) as pcv,
        tc.tile_pool(name="ps_tr", bufs=2, space=bass.MemorySpace.PSUM) as ptr,
    ):
        scratch = scrp.tile([128, SPAN1], BF16)
        blk_idx = 0
        prev = None    # (qkwin_tile, width) pending transposes

        def do_transposes(qkwin, width):
            # 4 transposes batched into one psum tile -> one copy out
            nonlocal blk_idx
            for j0 in range(0, width, 512):
                wj = min(512, width - j0)
                nb = wj // 128
                trps = ptr.tile([128, 512], BF16, tag="trps")
                for j in range(nb):
                    nc.tensor.transpose(trps[:, j * 128:(j + 1) * 128],
                                        qkwin[:, j0 + j * 128:j0 + (j + 1) * 128],
                                        w_idb[:])
                trsb = trsbp.tile([128, 512], BF16)
                nc.scalar.copy(trsb[:, 0:wj], trps[:, 0:wj])
                for j in range(nb):
                    mm(acc_ps[:], trsb[:, j * 128:j * 128 + C],
                       trsb[:, j * 128 + C:(j + 1) * 128],
                       start=(blk_idx == 0), stop=(blk_idx == n_blk_total - 1))
                    blk_idx += 1

        for wstart, width in windows:
            p_start = P0 + wstart
            ws = p_start - 259
            wwidth = width + 518
            xw = xw1p.tile([128, SPAN1 + 518], BF16)
            nc.gpsimd.dma_start(xw[0:C, 0:wwidth], x1[:, ws:ws + wwidth])
            nc.gpsimd.dma_start(xw[C:128, 0:wwidth], x2[:, ws:ws + wwidth])

            qkwin = qkwp.tile([128, SPAN1], BF16)
            # chunks in groups of 4 sharing each tap's LDWEIGHTS
            for g0 in range(0, width, 4 * CHUNK):
                lcs = [lc for lc in range(g0, min(g0 + 4 * CHUNK, width), CHUNK)]
                pss = [pcv.tile([128, CHUNK], f, tag="qkps", name=f"qkps{i}")
                       for i in range(len(lcs))]
                for t in range(9):
                    for qk_ps, lc in zip(pss, lcs):
                        L = min(CHUNK, width - lc)
                        o = lc + TAP_OFF[t]
                        mm(qk_ps[:, 0:L], w_lqk9[:, t * 128:(t + 1) * 128],
                           xw[:, o:o + L], start=(t == 0), stop=(t == 8))
                for qk_ps, lc in zip(pss, lcs):
                    L = min(CHUNK, width - lc)
                    nc.vector.tensor_copy(qkwin[:, lc:lc + L], qk_ps[:, 0:L])

            # zero the per-row pad columns so they don't pollute S / norms
            for off, ln in _pad_positions(p_start, width):
                nc.gpsimd.memset(qkwin[:, off:off + ln], 0.0)

            # norms: accumulate sum of squares over this window
            acc_tmp = acctp.tile([128, 1], f)
            nc.scalar.activation(
                scratch[:, 0:width], qkwin[:, 0:width],
                mybir.ActivationFunctionType.Square, accum_out=acc_tmp[:])
            nc.vector.tensor_add(qk2[:], qk2[:], acc_tmp[:])

            # software pipeline: transposes of the PREVIOUS window run after
            # this window's conv matmuls, keeping PE fed while copies drain
            if prev is not None:
                do_transposes(*prev)
            prev = (qkwin, width)

        do_transposes(*prev)

    # prefetch ALL pass-2 windows; DMA overlaps the pass-1 tail and the
    # collective below. Also load the mid-section weights now.
    for wi in range(n_win2):
        load_win2(wi)
    nc.gpsimd.dma_start(w_wv9[:], wv9[:])
    nc.gpsimd.dma_start(w_pT[:], pT[:])
    nc.gpsimd.dma_start(w_temp[:], temp64[:])
    nc.gpsimd.dma_start(w_id[:], ident[:])
    nc.gpsimd.dma_start(w_mask[:], mask64[:])

    # ---------------- collective: S and norms over the batch pair --------
    cc_sb = persist.tile([128, C + 1], f)
    nc.vector.memset(cc_sb[:], 0.0)
    nc.scalar.copy(cc_sb[0:C, 0:C], acc_ps[:])
    nc.vector.tensor_copy(cc_sb[:, C:C + 1], qk2[:])
    cc_in = dram.tile([128, C + 1], f)
    cc_out = dram.tile([128, C + 1], f, tag="cc_out")
    nc.sync.dma_start(cc_in[:], cc_sb[:])
    nc.gpsimd.collective_compute(
        "AllReduce", mybir.AluOpType.add,
        replica_groups=[[0, 1], [2, 3], [4, 5], [6, 7]],
        ins=[cc_in.opt()], outs=[cc_out.opt()],
    )
    sqk = persist.tile([128, C + 1], f, tag="sqk")
    nc.sync.dma_start(sqk[:], cc_out[:])

    # ---------------- tiny mid-section: softmax, M^T, G^T, pass-2 lhsT ---
    g6 = []   # 6 bf16 [128,128] lhsT tiles for pass 2: (parity, dx)
    with (
        tc.tile_pool(name="mid", bufs=1) as midp,
        tc.tile_pool(name="ps_mid", bufs=1, space=bass.MemorySpace.PSUM) as pmid,
    ):
        nrm = midp.tile([128, 1], f, tag="nrm")       # sqrt of sums
        nc.scalar.sqrt(nrm[:], sqk[:, C:C + 1])
        nc.vector.tensor_scalar_max(nrm[:], nrm[:], EPS)
        rn = midp.tile([128, 1], f, tag="rn")         # 1/||.||
        nc.vector.reciprocal(rn[:], nrm[:])
        rs = midp.tile([C, 1], f, tag="rs")           # temp/||q|| per row c
        nc.vector.tensor_mul(rs[:], rn[0:C, :], w_temp[:])

        # broadcast 1/||k|| along free dim: transpose then rank-1 outer
        nkT_ps = pmid.tile([1, C], f, tag="nkT")
        nc.tensor.transpose(nkT_ps[:], rn[C:128, :], w_id[C:128, C:128])
        nkT = midp.tile([1, C], f, tag="nkT_sb")
        nc.scalar.copy(nkT[:], nkT_ps[:])
        ones1 = midp.tile([1, C], f, tag="ones1")
        nc.vector.memset(ones1[:], 1.0)
        nkb_ps = pmid.tile([C, C], f, tag="nkb")
        mm(nkb_ps[:], ones1[:], nkT[:])
        # logits = S * rs(row) * (1/||k||)(col)
        sp = midp.tile([C, C], f, tag="sp")
        nc.vector.tensor_scalar(sp[:], sqk[0:C, 0:C], rs[:], None,
                                op0=mybir.AluOpType.mult)
        nc.vector.tensor_mul(sp[:], sp[:], nkb_ps[:])

        # blockwise softmax via additive off-block mask (-1e30):
        # off-block entries exp to exactly 0, so the result IS Ablk.
        nc.vector.tensor_add(sp[:], sp[:], w_mask[:])
        negm = midp.tile([C, 1], f, tag="negm")
        nc.vector.tensor_reduce(negm[:], sp[:], axis=mybir.AxisListType.X,
                                op=mybir.AluOpType.max, negate=True)
        den = midp.tile([C, 1], f, tag="den")
        ex = midp.tile([C, C], f, tag="ex")
        nc.scalar.activation(ex[:], sp[:], mybir.ActivationFunctionType.Exp,
                             bias=negm[:], scale=1.0, accum_out=den[:])
        rden = midp.tile([C, 1], f, tag="rden")
        nc.vector.reciprocal(rden[:], den[:])
        ablk = midp.tile([C, C], f, tag="ablk")
        nc.vector.tensor_scalar(ablk[:], ex[:], rden[:], None,
                                op0=mybir.AluOpType.mult)

        # M^T = Ablk^T @ P^T
        mt_ps = pmid.tile([C, C], f, tag="mt")
        mm(mt_ps[:], ablk[:], w_pT[:])
        mt = midp.tile([C, C], f, tag="mt_sb")
        nc.scalar.copy(mt[:], mt_ps[:])

        # G^T[dp] = wv9[dp] @ M^T, computed into both partition halves so
        # the 6 pass-2 lhsT tiles can be assembled with lane-aligned copies
        gt_ps = pmid.tile([128, 9 * C], f, tag="gt")
        for dp in range(9):
            s = dp * C
            mm(gt_ps[0:C, s:s + C], w_wv9[:, s:s + C], mt[:],
               tile_position=(0, 0))
            mm(gt_ps[C:128, s:s + C], w_wv9[:, s:s + C], mt[:],
               tile_position=(0, 64))

        # pass-2 stationary tiles:
        #   A(dx) = [[G(-1,dx)^T, 0         ], [G(0,dx)^T,  G(-1,dx)^T]]
        #   B(dx) = [[G(+1,dx)^T, G(0,dx)^T ], [0,          G(+1,dx)^T]]
        # (quadrants as [k-range 0:64 | 64:128] x [m-range 0:64 | 64:128])
        def tidx(dy, dx):
            return ((dy + 1) * 3 + (dx + 1)) * C

        for dx in (-1, 0, 1):
            ga = persist.tile([128, 128], BF16, tag=f"gA{dx}")
            nc.vector.memset(ga[:], 0.0)
            sm1, s0 = tidx(-1, dx), tidx(0, dx)
            nc.scalar.copy(ga[0:C, 0:C], gt_ps[0:C, sm1:sm1 + C])
            nc.scalar.copy(ga[C:128, 0:C], gt_ps[C:128, s0:s0 + C])
            nc.scalar.copy(ga[C:128, C:128], gt_ps[C:128, sm1:sm1 + C])
            gb = persist.tile([128, 128], BF16, tag=f"gB{dx}")
            nc.vector.memset(gb[:], 0.0)
            sp1 = tidx(1, dx)
            nc.scalar.copy(gb[0:C, 0:C], gt_ps[0:C, sp1:sp1 + C])
            nc.scalar.copy(gb[0:C, C:128], gt_ps[0:C, s0:s0 + C])
            nc.scalar.copy(gb[C:128, C:128], gt_ps[C:128, sp1:sp1 + C])
            g6.append((ga, gb))

    accp.release()

    # ---------------- pass 2: out = G-conv(x2) via row pairs, write y ----
    with (
        tc.tile_pool(name="osb", bufs=4) as osbp,
        tc.tile_pool(name="ps_p2", bufs=4, space=bass.MemorySpace.PSUM) as pp2,
    ):
        for wi in range(n_win2):
            base_pair = wi * PAIRS2
            np_w = min(PAIRS2, n_pairs - base_pair)
            xw = xw2_tiles.pop(wi)
            # pairs in groups of 4 sharing each lhsT's LDWEIGHTS
            for pg in range(0, np_w, 4):
                pls = list(range(pg, min(pg + 4, np_w)))
                pss = [pp2.tile([128, R], f, tag="o2", name=f"o2_{i}")
                       for i in range(len(pls))]
                for i, dx in enumerate((-1, 0, 1)):
                    ga, gb = g6[i]
                    for ps, p in zip(pss, pls):
                        oA = (2 * p) * R + dx + 1
                        mm(ps[:], ga[:], xw[:, oA:oA + R],
                           start=(i == 0), stop=False)
                    for ps, p in zip(pss, pls):
                        oB = (2 * p + 2) * R + dx + 1
                        mm(ps[:], gb[:], xw[:, oB:oB + R],
                           start=False, stop=(i == 2))
                for ps, p in zip(pss, pls):
                    obs = osbp.tile([128, R], BF16)
                    nc.vector.tensor_copy(obs[:], ps[:])
                    row = 2 * (base_pair + p)
                    nc.sync.dma_start(y[:, row, :], obs[0:C, 1:W + 1])
                    nc.sync.dma_start(y[:, row + 1, :], obs[C:128, 1:W + 1])

    xw2p.release()
    for p in (persist, dram, wpool):
        p.release()


# ======================= host side =========================================

def _prep_consts(q_w, q_dw_w, kv_w, kv_dw_w, proj_w, temperature):
    q_w = np.asarray(q_w, np.float32)[:, :, 0, 0]          # [o, i]
    kv_w = np.asarray(kv_w, np.float32)[:, :, 0, 0]        # [2C, i]
    q_dw = np.asarray(q_dw_w, np.float32)[:, 0]            # [C, 3, 3]
    kv_dw = np.asarray(kv_dw_w, np.float32)[:, 0]          # [2C, 3, 3]
    proj = np.asarray(proj_w, np.float32)[:, :, 0, 0]      # [o, c]
    temp = np.asarray(temperature, np.float32).reshape(HEADS)

    lqk9 = np.zeros((128, 9 * 128), np.float32)
    wv9 = np.zeros((C, 9 * C), np.float32)
    for t, (dy, dx) in enumerate(TAPS):
        w9q = q_dw[:, dy + 1, dx + 1][:, None] * q_w       # [o, i]
        w9k = kv_dw[0:C, dy + 1, dx + 1][:, None] * kv_w[0:C]
        lqk9[0:C, t * 128:t * 128 + C] = w9q.T
        lqk9[C:128, t * 128 + C:(t + 1) * 128] = w9k.T
        # wv9[dp][d, i] = wdv[d, dp] * Wv[d, i]
        wv9[:, t * C:(t + 1) * C] = (
            kv_dw[C:2 * C, dy + 1, dx + 1][:, None] * kv_w[C:2 * C]
        )
    pTm = proj.T.copy()                                    # [c, o]
    temp64 = np.repeat(temp, HD).reshape(C, 1).astype(np.float32)
    ident = np.eye(128, dtype=np.float32)
    identb = np.eye(128, dtype=NP_BF16)
    mask = np.full((C, C), -1e30, np.float32)
    for h in range(HEADS):
        mask[h * HD:(h + 1) * HD, h * HD:(h + 1) * HD] = 0.0
    return lqk9.astype(NP_BF16), wv9, pTm, temp64, ident, identb, mask


def _prep_slices(img_bf):
    """[C, H, W] bf16 -> padded flat [C, NBUF] per half; returns (top, bot)."""
    out = []
    for h in range(2):
        buf = np.zeros((C, NR, R), NP_BF16)
        r0 = h * HH
        lo, hi = r0 - 1, r0 + HH + 1
        vlo, vhi = max(lo, 0), min(hi, H)
        buf[:, 1 + (vlo - lo):1 + (vlo - lo) + (vhi - vlo), 1:W + 1] = \
            img_bf[:, vlo:vhi, :]
        out.append(np.ascontiguousarray(buf.reshape(C, NBUF)))
    return out


def _build_in_maps(input1, input2, q_w, q_dw_w, kv_w, kv_dw_w, proj_w,
                   temperature):
    lqk9, wv9, pTm, temp64, ident, identb, mask = _prep_consts(
        q_w, q_dw_w, kv_w, kv_dw_w, proj_w, temperature)
    in1_bf = np.asarray(input1, np.float32).astype(NP_BF16)
    in2_bf = np.asarray(input2, np.float32).astype(NP_BF16)
    in_maps = []
    for core in range(8):
        b, h = core // 2, core % 2
        x1t = _prep_slices(in1_bf[b])[h]
        x2t = _prep_slices(in2_bf[b])[h]
        in_maps.append({
            "x1": x1t, "x2": x2t, "lqk9": lqk9, "wv9": wv9, "pT": pTm,
            "temp64": temp64, "ident": ident, "identb": identb,
            "mask64": mask,
        })
    return in_maps


def kernel(input1, input2, q_w, q_dw_w, kv_w, kv_dw_w, proj_w, temperature):
    if "nc" not in _CACHE:
        _CACHE["nc"] = build_module()
    nc = _CACHE["nc"]

    in_maps = _build_in_maps(input1, input2, q_w, q_dw_w, kv_w, kv_dw_w,
                             proj_w, temperature)
    results = _get_runner(nc)(in_maps)
    out = np.empty((B, C, H, W), np.float32)
    for core in range(8):
        b, h = core // 2, core % 2
        out[b, :, h * HH:(h + 1) * HH, :] = \
            results[core]["y"].astype(np.float32)
    return out


def _get_runner(nc, n_cores=8):
    """Like bass2jax.run_bass_via_pjrt, but the jitted shard_map executable is
    built once and reused across calls (avoids per-call retracing)."""
    if "runner" in _CACHE:
        return _CACHE["runner"]
    import jax
    from jax.sharding import Mesh, PartitionSpec
    from jax.experimental.shard_map import shard_map
    from concourse import bass2jax as b2j
    from concourse import mybir as _mb

    b2j.install_neuronx_cc_hook()
    partition_name = nc.partition_id_tensor.name if nc.partition_id_tensor else None
    in_names, out_names, out_avals, zero_shapes = [], [], [], []
    for alloc in nc.m.functions[0].allocations:
        if not isinstance(alloc, _mb.MemoryLocationSet):
            continue
        name = alloc.memorylocations[0].name
        if alloc.kind == "ExternalInput":
            if name != partition_name:
                in_names.append(name)
        elif alloc.kind == "ExternalOutput":
            out_names.append(name)
            shape = tuple(alloc.tensor_shape)
            dtype = _mb.dt.np(alloc.dtype)
            out_avals.append(jax.core.ShapedArray(shape, dtype))
            zero_shapes.append((shape, dtype))
    n_params = len(in_names)
    n_outs = len(out_avals)
    all_in_names = list(in_names) + list(out_names)
    if partition_name is not None:
        all_in_names.append(partition_name)
    donate = tuple(range(n_params, n_params + n_outs))

    def _pjrt_body(*args):
        operands = list(args)
        if partition_name is not None:
            operands.append(b2j.partition_id_tensor())
        return tuple(b2j._bass_exec_p.bind(
            *operands, out_avals=tuple(out_avals), in_names=tuple(all_in_names),
            out_names=tuple(out_names), lowering_input_output_aliases=(),
            sim_require_finite=True, sim_require_nnan=True, nc=nc))

    devices = jax.devices()[:n_cores]
    mesh = Mesh(np.asarray(devices), ("core",))
    sharded = jax.jit(
        shard_map(_pjrt_body, mesh=mesh,
                  in_specs=(PartitionSpec("core"),) * (n_params + n_outs),
                  out_specs=(PartitionSpec("core"),) * n_outs, check_rep=False),
        donate_argnums=donate, keep_unused=True)

    def run(in_maps):
        concat_in = [
            np.concatenate([np.asarray(in_maps[c][nm]) for c in range(n_cores)], 0)
            for nm in in_names
        ]
        concat_zeros = [np.zeros((n_cores * s[0], *s[1:]), d)
                        for s, d in zero_shapes]
        out_arrs = sharded(*concat_in, *concat_zeros)
        return [
            {nm: np.asarray(out_arrs[i]).reshape(n_cores, *out_avals[i].shape)[c]
             for i, nm in enumerate(out_names)}
            for c in range(n_cores)
        ]

    _CACHE["runner"] = run
    return run


# revision 18
# speedup vs baseline: 25048.2980x; 1.3096x over previous
"""CAB (channel-attention block) Trainium2 kernel, bf16 datapath.

Sharding: 8 cores = 4 batches x 2 H-halves. Each core computes its
[64, 128, 256] slice of the output. The q.kT contraction and the L2-norm
sums are AllReduced across the 2 cores sharing a batch (sequence-parallel).

Math folds (all exact):
  - L2 normalize folds into S: attn_logits = S * temp / (||q|| ||k||^T),
    with S = q @ k^T computed on raw (unnormalized) q, k.
  - proj o (attn @ v) o dwconv_v o pwconv_v folds into a single 3x3 dense
    conv on input2 with data-dependent matrices
    G[dp] = (P @ A_blockdiag) @ (diag(wdv[:,dp]) @ Wv).

PE packing:
  - pass 1: q-conv (x1) and k-conv (x2) run as ONE matmul per tap with a
    block-diagonal [128,128] lhsT and rhs = [x1; x2] stacked on partitions
    (K=128, M=128) -> 9 PE cycles/col for both convs.
  - pass 2: rhs = [x2 @ off; x2 @ off+R] (row-shifted copies on the two
    partition halves); each [128,128] lhsT computes TWO output rows at
    once -> 6 matmuls per row-pair (3 PE cycles/col).
"""
import sys

sys.path.insert(0, "/opt/trn_rl_repo")

import numpy as np
import ml_dtypes

import concourse.bacc as bacc
import concourse.bass as bass
import concourse.tile as tile
from concourse import mybir
from concourse.bass_utils import run_bass_kernel_spmd

F32 = mybir.dt.float32
BF16 = mybir.dt.bfloat16
NP_BF16 = ml_dtypes.bfloat16

B, C, H, W = 4, 64, 256, 256
HEADS = 8
HD = C // HEADS
EPS = 1e-12

HH = H // 2            # rows per core
R = W + 2              # padded row length
NR = HH + 4            # buffer rows: zero, halo, 128 data, halo, zero
NBUF = NR * R          # per-core padded input length (flattened)
P0 = 2 * R             # first output position (data row 0, col -1(pad))
NOUT = HH * R          # output span incl. per-row col pads

TAPS = [(dy, dx) for dy in (-1, 0, 1) for dx in (-1, 0, 1)]
# rhs offset of tap within a window that starts 259 cols before the chunk
TAP_OFF = [259 + dy * R + dx for dy, dx in TAPS]

SPAN1 = 4096           # pass-1 window span (multiple of 128)
CHUNK = 512            # pass-1 psum chunk (one bank)
PAIRS2 = 8             # pass-2 row-pairs per window

_CACHE = {}


def _pad_positions(start, length):
    """Contiguous runs of pad columns (global col % R in {0, R-1}) within
    [start, start+length), as (offset_rel, run_len) with runs <= 2."""
    runs = []
    end = start + length
    p = (start // R) * R - 1
    while p < end:
        for q in (p, p + 1):  # (row r col 257, row r+1 col 0) adjacent pair
            lo = max(q, start)
            hi = min(q + 1, end)
            if lo < hi:
                if runs and runs[-1][0] + runs[-1][1] == lo - start:
                    runs[-1] = (runs[-1][0], runs[-1][1] + (hi - lo))
                else:
                    runs.append((lo - start, hi - lo))
        p += R
    return runs


def build_module():
    nc = bacc.Bacc("TRN2", target_bir_lowering=False, debug=False, num_devices=8)

    x1 = nc.declare_dram_parameter("x1", [C, NBUF], BF16, isOutput=False)
    x2 = nc.declare_dram_parameter("x2", [C, NBUF], BF16, isOutput=False)
    lqk9 = nc.declare_dram_parameter("lqk9", [128, 9 * 128], BF16, isOutput=False)
    wv9 = nc.declare_dram_parameter("wv9", [C, 9 * C], F32, isOutput=False)
    pT = nc.declare_dram_parameter("pT", [C, C], F32, isOutput=False)
    temp64 = nc.declare_dram_parameter("temp64", [C, 1], F32, isOutput=False)
    ident = nc.declare_dram_parameter("ident", [128, 128], F32, isOutput=False)
    identb = nc.declare_dram_parameter("identb", [128, 128], BF16, isOutput=False)
    mask64 = nc.declare_dram_parameter("mask64", [C, C], F32, isOutput=False)
    y = nc.declare_dram_parameter("y", [C, HH, W], BF16, isOutput=True)

    with tile.TileContext(nc) as tc:
        _body(tc, nc, x1, x2, lqk9, wv9, pT, temp64, ident, identb, mask64, y)
    nc.compile()
    return nc


def _body(tc, nc, x1, x2, lqk9, wv9, pT, temp64, ident, identb, mask64, y):
    mm = nc.tensor.matmul
    f = F32

    wpool = tc.alloc_tile_pool(name="weights", bufs=1)
    dram = tc.alloc_tile_pool(name="dram", bufs=1, space="DRAM")
    accp = tc.alloc_tile_pool(name="ps_acc", bufs=1, space=bass.MemorySpace.PSUM)
    persist = tc.alloc_tile_pool(name="persist", bufs=1)

    # only the pass-1 weights load up-front; mid-section weights load later
    # so the first x-window DMA isn't queued behind them
    w_lqk9 = wpool.tile([128, 9 * 128], BF16)
    nc.gpsimd.dma_start(w_lqk9[:], lqk9[:])
    w_idb = wpool.tile([128, 128], BF16)
    nc.gpsimd.dma_start(w_idb[:], identb[:])
    w_wv9 = wpool.tile([C, 9 * C], f)
    w_pT = wpool.tile([C, C], f)
    w_temp = wpool.tile([C, 1], f)
    w_id = wpool.tile([128, 128], f)
    w_mask = wpool.tile([C, C], f)

    acc_ps = accp.tile([C, C], f)          # S accumulator (q.kT)
    qk2 = persist.tile([128, 1], f)        # running sum q^2 (top) / k^2 (bottom)
    nc.vector.memset(qk2[:], 0.0)

    # pass-2 window prefetch machinery (tiles loaded before the collective
    # so the DMA overlaps the AllReduce latency)
    n_pairs = HH // 2
    n_win2 = (n_pairs + PAIRS2 - 1) // PAIRS2
    xw2p = tc.alloc_tile_pool(name="xw2", bufs=n_win2)
    xw2_tiles = {}

    W2COLS = (2 * PAIRS2 + 1) * R

    def load_win2(wi):
        # partitions 0:64 <- x2 rows (rr-1)..(rr+15); 64:128 <- one row down
        rr = 2 * wi * PAIRS2
        s0 = (rr + 1) * R
        xw = xw2p.tile([128, W2COLS], BF16)
        nc.gpsimd.dma_start(xw[0:C, :], x2[:, s0:s0 + W2COLS])
        nc.gpsimd.dma_start(xw[C:128, :], x2[:, s0 + R:s0 + R + W2COLS])
        xw2_tiles[wi] = xw

    # ---------------- pass 1: q,k conv -> transpose -> S, norms ----------
    n_blk_total = NOUT // 128
    windows = []
    ws_ = 0
    first_span = 1024          # small first window -> PE starts sooner
    while ws_ < NOUT:
        span = first_span if ws_ == 0 else SPAN1
        windows.append((ws_, min(span, NOUT - ws_)))
        ws_ += windows[-1][1]

    with (
        tc.tile_pool(name="xw1", bufs=2) as xw1p,
        tc.tile_pool(name="qkwin", bufs=2) as qkwp,
        tc.tile_pool(name="trsb", bufs=3) as trsbp,
        tc.tile_pool(name="scratch", bufs=1) as scrp,
        tc.tile_pool(name="acct", bufs=2) as acctp,
        tc.tile_pool(name="ps_conv", bufs=2, space=bass.MemorySpace.PSUM) as pcv,
        tc.tile_pool(name="ps_tr", bufs=2, space=bass.MemorySpace.PSUM) as ptr,
    ):
        scratch = scrp.tile([128, SPAN1], BF16)
        blk_idx = 0
        prev = None    # (qkwin_tile, width) pending transposes

        def do_transposes(qkwin, width):
            # 4 transposes batched into one psum tile -> one copy out
            nonlocal blk_idx
            for j0 in range(0, width, 512):
                wj = min(512, width - j0)
                nb = wj // 128
                trps = ptr.tile([128, 512], BF16, tag="trps")
                for j in range(nb):
                    nc.tensor.transpose(trps[:, j * 128:(j + 1) * 128],
                                        qkwin[:, j0 + j * 128:j0 + (j + 1) * 128],
                                        w_idb[:])
                trsb = trsbp.tile([128, 512], BF16)
                nc.scalar.copy(trsb[:, 0:wj], trps[:, 0:wj])
                for j in range(nb):
                    mm(acc_ps[:], trsb[:, j * 128:j * 128 + C],
                       trsb[:, j * 128 + C:(j + 1) * 128],
                       start=(blk_idx == 0), stop=(blk_idx == n_blk_total - 1))
                    blk_idx += 1

        for wstart, width in windows:
            p_start = P0 + wstart
            ws = p_start - 259
            wwidth = width + 518
            xw = xw1p.tile([128, SPAN1 + 518], BF16)
            nc.gpsimd.dma_start(xw[0:C, 0:wwidth], x1[:, ws:ws + wwidth])
            nc.gpsimd.dma_start(xw[C:128, 0:wwidth], x2[:, ws:ws + wwidth])

            qkwin = qkwp.tile([128, SPAN1], BF16)
            for lc in range(0, width, CHUNK):
                L = min(CHUNK, width - lc)
                qk_ps = pcv.tile([128, CHUNK], f, tag="qkps")
                for t in range(9):
                    o = lc + TAP_OFF[t]
                    mm(qk_ps[:, 0:L], w_lqk9[:, t * 128:(t + 1) * 128],
                       xw[:, o:o + L], start=(t == 0), stop=(t == 8))
                nc.vector.tensor_copy(qkwin[:, lc:lc + L], qk_ps[:, 0:L])

            # zero the per-row pad columns so they don't pollute S / norms
            for off, ln in _pad_positions(p_start, width):
                nc.gpsimd.memset(qkwin[:, off:off + ln], 0.0)

            # norms: accumulate sum of squares over this window
            acc_tmp = acctp.tile([128, 1], f)
            nc.scalar.activation(
                scratch[:, 0:width], qkwin[:, 0:width],
                mybir.ActivationFunctionType.Square, accum_out=acc_tmp[:])
            nc.vector.tensor_add(qk2[:], qk2[:], acc_tmp[:])

            # software pipeline: transposes of the PREVIOUS window run after
            # this window's conv matmuls, keeping PE fed while copies drain
            if prev is not None:
                do_transposes(*prev)
            prev = (qkwin, width)

        do_transposes(*prev)

    # prefetch ALL pass-2 windows; DMA overlaps the pass-1 tail and the
    # collective below. Also load the mid-section weights now.
    for wi in range(n_win2):
        load_win2(wi)
    nc.gpsimd.dma_start(w_wv9[:], wv9[:])
    nc.gpsimd.dma_start(w_pT[:], pT[:])
    nc.gpsimd.dma_start(w_temp[:], temp64[:])
    nc.gpsimd.dma_start(w_id[:], ident[:])
    nc.gpsimd.dma_start(w_mask[:], mask64[:])

    # ---------------- collective: S and norms over the batch pair --------
    cc_sb = persist.tile([128, C + 1], f)
    nc.vector.memset(cc_sb[:], 0.0)
    nc.scalar.copy(cc_sb[0:C, 0:C], acc_ps[:])
    nc.vector.tensor_copy(cc_sb[:, C:C + 1], qk2[:])
    cc_in = dram.tile([128, C + 1], f)
    cc_out = dram.tile([128, C + 1], f, tag="cc_out")
    nc.sync.dma_start(cc_in[:], cc_sb[:])
    nc.gpsimd.collective_compute(
        "AllReduce", mybir.AluOpType.add,
        replica_groups=[[0, 1], [2, 3], [4, 5], [6, 7]],
        ins=[cc_in.opt()], outs=[cc_out.opt()],
    )
    sqk = persist.tile([128, C + 1], f, tag="sqk")
    nc.sync.dma_start(sqk[:], cc_out[:])

    # ---------------- tiny mid-section: softmax, M^T, G^T, pass-2 lhsT ---
    g6 = []   # 6 bf16 [128,128] lhsT tiles for pass 2: (parity, dx)
    with (
        tc.tile_pool(name="mid", bufs=1) as midp,
        tc.tile_pool(name="ps_mid", bufs=1, space=bass.MemorySpace.PSUM) as pmid,
    ):
        nrm = midp.tile([128, 1], f, tag="nrm")       # sqrt of sums
        nc.scalar.sqrt(nrm[:], sqk[:, C:C + 1])
        nc.vector.tensor_scalar_max(nrm[:], nrm[:], EPS)
        rn = midp.tile([128, 1], f, tag="rn")         # 1/||.||
        nc.vector.reciprocal(rn[:], nrm[:])
        rs = midp.tile([C, 1], f, tag="rs")           # temp/||q|| per row c
        nc.vector.tensor_mul(rs[:], rn[0:C, :], w_temp[:])

        # broadcast 1/||k|| along free dim: transpose then rank-1 outer
        nkT_ps = pmid.tile([1, C], f, tag="nkT")
        nc.tensor.transpose(nkT_ps[:], rn[C:128, :], w_id[C:128, C:128])
        nkT = midp.tile([1, C], f, tag="nkT_sb")
        nc.scalar.copy(nkT[:], nkT_ps[:])
        ones1 = midp.tile([1, C], f, tag="ones1")
        nc.vector.memset(ones1[:], 1.0)
        nkb_ps = pmid.tile([C, C], f, tag="nkb")
        mm(nkb_ps[:], ones1[:], nkT[:])
        # logits = S * rs(row) * (1/||k||)(col)
        sp = midp.tile([C, C], f, tag="sp")
        nc.vector.tensor_scalar(sp[:], sqk[0:C, 0:C], rs[:], None,
                                op0=mybir.AluOpType.mult)
        nc.vector.tensor_mul(sp[:], sp[:], nkb_ps[:])

        # blockwise softmax via additive off-block mask (-1e30):
        # off-block entries exp to exactly 0, so the result IS Ablk.
        nc.vector.tensor_add(sp[:], sp[:], w_mask[:])
        negm = midp.tile([C, 1], f, tag="negm")
        nc.vector.tensor_reduce(negm[:], sp[:], axis=mybir.AxisListType.X,
                                op=mybir.AluOpType.max, negate=True)
        den = midp.tile([C, 1], f, tag="den")
        ex = midp.tile([C, C], f, tag="ex")
        nc.scalar.activation(ex[:], sp[:], mybir.ActivationFunctionType.Exp,
                             bias=negm[:], scale=1.0, accum_out=den[:])
        rden = midp.tile([C, 1], f, tag="rden")
        nc.vector.reciprocal(rden[:], den[:])
        ablk = midp.tile([C, C], f, tag="ablk")
        nc.vector.tensor_scalar(ablk[:], ex[:], rden[:], None,
                                op0=mybir.AluOpType.mult)

        # M^T = Ablk^T @ P^T
        mt_ps = pmid.tile([C, C], f, tag="mt")
        mm(mt_ps[:], ablk[:], w_pT[:])
        mt = midp.tile([C, C], f, tag="mt_sb")
        nc.scalar.copy(mt[:], mt_ps[:])

        # G^T[dp] = wv9[dp] @ M^T, computed into both partition halves so
        # the 6 pass-2 lhsT tiles can be assembled with lane-aligned copies
        gt_ps = pmid.tile([128, 9 * C], f, tag="gt")
        for dp in range(9):
            s = dp * C
            mm(gt_ps[0:C, s:s + C], w_wv9[:, s:s + C], mt[:],
               tile_position=(0, 0))
            mm(gt_ps[C:128, s:s + C], w_wv9[:, s:s + C], mt[:],
               tile_position=(0, 64))

        # pass-2 stationary tiles:
        #   A(dx) = [[G(-1,dx)^T, 0         ], [G(0,dx)^T,  G(-1,dx)^T]]
        #   B(dx) = [[G(+1,dx)^T, G(0,dx)^T ], [0,          G(+1,dx)^T]]
        # (quadrants as [k-range 0:64 | 64:128] x [m-range 0:64 | 64:128])
        def tidx(dy, dx):
            return ((dy + 1) * 3 + (dx + 1)) * C

        for dx in (-1, 0, 1):
            ga = persist.tile([128, 128], BF16, tag=f"gA{dx}")
            nc.vector.memset(ga[:], 0.0)
            sm1, s0 = tidx(-1, dx), tidx(0, dx)
            nc.scalar.copy(ga[0:C, 0:C], gt_ps[0:C, sm1:sm1 + C])
            nc.scalar.copy(ga[C:128, 0:C], gt_ps[C:128, s0:s0 + C])
            nc.scalar.copy(ga[C:128, C:128], gt_ps[C:128, sm1:sm1 + C])
            gb = persist.tile([128, 128], BF16, tag=f"gB{dx}")
            nc.vector.memset(gb[:], 0.0)
            sp1 = tidx(1, dx)
            nc.scalar.copy(gb[0:C, 0:C], gt_ps[0:C, sp1:sp1 + C])
            nc.scalar.copy(gb[0:C, C:128], gt_ps[0:C, s0:s0 + C])
            nc.scalar.copy(gb[C:128, C:128], gt_ps[C:128, sp1:sp1 + C])
            g6.append((ga, gb))

    accp.release()

    # ---------------- pass 2: out = G-conv(x2) via row pairs, write y ----
    with (
        tc.tile_pool(name="osb", bufs=4) as osbp,
        tc.tile_pool(name="ps_p2", bufs=4, space=bass.MemorySpace.PSUM) as pp2,
    ):
        for wi in range(n_win2):
            base_pair = wi * PAIRS2
            np_w = min(PAIRS2, n_pairs - base_pair)
            xw = xw2_tiles.pop(wi)
            pdim = list(xw.ap)[0]          # [stride, 128] partition dim

            def rhs2(col):                 # 2 row-pairs, 256 cols each
                return bass.AP(xw.tensor, xw.offset + col,
                               [pdim, [2 * R, 2], [1, W]])

            # two row-pairs (4 output rows) per matmul: N=512, one bank
            for p in range(0, np_w, 2):
                ps3 = pp2.tile([128, 2, W], f, tag="o2")
                for i, dx in enumerate((-1, 0, 1)):
                    ga, gb = g6[i]
                    mm(ps3[:], ga[:], rhs2((2 * p) * R + dx + 1),
                       start=(i == 0), stop=False)
                    mm(ps3[:], gb[:], rhs2((2 * p + 2) * R + dx + 1),
                       start=False, stop=(i == 2))
                obs3 = osbp.tile([128, 2, W], BF16)
                nc.vector.tensor_copy(obs3[:], ps3[:])
                row = 2 * (base_pair + p)
                nc.sync.dma_start(y[:, row:row + 4:2, :], obs3[0:C])
                nc.sync.dma_start(y[:, row + 1:row + 4:2, :], obs3[C:128])

    xw2p.release()
    for p in (persist, dram, wpool):
        p.release()


# ======================= host side =========================================

def _prep_consts(q_w, q_dw_w, kv_w, kv_dw_w, proj_w, temperature):
    q_w = np.asarray(q_w, np.float32)[:, :, 0, 0]          # [o, i]
    kv_w = np.asarray(kv_w, np.float32)[:, :, 0, 0]        # [2C, i]
    q_dw = np.asarray(q_dw_w, np.float32)[:, 0]            # [C, 3, 3]
    kv_dw = np.asarray(kv_dw_w, np.float32)[:, 0]          # [2C, 3, 3]
    proj = np.asarray(proj_w, np.float32)[:, :, 0, 0]      # [o, c]
    temp = np.asarray(temperature, np.float32).reshape(HEADS)

    lqk9 = np.zeros((128, 9 * 128), np.float32)
    wv9 = np.zeros((C, 9 * C), np.float32)
    for t, (dy, dx) in enumerate(TAPS):
        w9q = q_dw[:, dy + 1, dx + 1][:, None] * q_w       # [o, i]
        w9k = kv_dw[0:C, dy + 1, dx + 1][:, None] * kv_w[0:C]
        lqk9[0:C, t * 128:t * 128 + C] = w9q.T
        lqk9[C:128, t * 128 + C:(t + 1) * 128] = w9k.T
        # wv9[dp][d, i] = wdv[d, dp] * Wv[d, i]
        wv9[:, t * C:(t + 1) * C] = (
            kv_dw[C:2 * C, dy + 1, dx + 1][:, None] * kv_w[C:2 * C]
        )
    pTm = proj.T.copy()                                    # [c, o]
    temp64 = np.repeat(temp, HD).reshape(C, 1).astype(np.float32)
    ident = np.eye(128, dtype=np.float32)
    identb = np.eye(128, dtype=NP_BF16)
    mask = np.full((C, C), -1e30, np.float32)
    for h in range(HEADS):
        mask[h * HD:(h + 1) * HD, h * HD:(h + 1) * HD] = 0.0
    return lqk9.astype(NP_BF16), wv9, pTm, temp64, ident, identb, mask


def _prep_slices(img_bf):
    """[C, H, W] bf16 -> padded flat [C, NBUF] per half; returns (top, bot)."""
    out = []
    for h in range(2):
        buf = np.zeros((C, NR, R), NP_BF16)
        r0 = h * HH
        lo, hi = r0 - 1, r0 + HH + 1
        vlo, vhi = max(lo, 0), min(hi, H)
        buf[:, 1 + (vlo - lo):1 + (vlo - lo) + (vhi - vlo), 1:W + 1] = \
            img_bf[:, vlo:vhi, :]
        out.append(np.ascontiguousarray(buf.reshape(C, NBUF)))
    return out


def _build_in_maps(input1, input2, q_w, q_dw_w, kv_w, kv_dw_w, proj_w,
                   temperature):
    lqk9, wv9, pTm, temp64, ident, identb, mask = _prep_consts(
        q_w, q_dw_w, kv_w, kv_dw_w, proj_w, temperature)
    in1_bf = np.asarray(input1, np.float32).astype(NP_BF16)
    in2_bf = np.asarray(input2, np.float32).astype(NP_BF16)
    in_maps = []
    for core in range(8):
        b, h = core // 2, core % 2
        x1t = _prep_slices(in1_bf[b])[h]
        x2t = _prep_slices(in2_bf[b])[h]
        in_maps.append({
            "x1": x1t, "x2": x2t, "lqk9": lqk9, "wv9": wv9, "pT": pTm,
            "temp64": temp64, "ident": ident, "identb": identb,
            "mask64": mask,
        })
    return in_maps


def kernel(input1, input2, q_w, q_dw_w, kv_w, kv_dw_w, proj_w, temperature):
    if "nc" not in _CACHE:
        _CACHE["nc"] = build_module()
    nc = _CACHE["nc"]

    in_maps = _build_in_maps(input1, input2, q_w, q_dw_w, kv_w, kv_dw_w,
                             proj_w, temperature)
    results = _get_runner(nc)(in_maps)
    out = np.empty((B, C, H, W), np.float32)
    for core in range(8):
        b, h = core // 2, core % 2
        out[b, :, h * HH:(h + 1) * HH, :] = \
            results[core]["y"].astype(np.float32)
    return out


def _get_runner(nc, n_cores=8):
    """Like bass2jax.run_bass_via_pjrt, but the jitted shard_map executable is
    built once and reused across calls (avoids per-call retracing)."""
    if "runner" in _CACHE:
        return _CACHE["runner"]
    import jax
    from jax.sharding import Mesh, PartitionSpec
    from jax.experimental.shard_map import shard_map
    from concourse import bass2jax as b2j
    from concourse import mybir as _mb

    b2j.install_neuronx_cc_hook()
    partition_name = nc.partition_id_tensor.name if nc.partition_id_tensor else None
    in_names, out_names, out_avals, zero_shapes = [], [], [], []
    for alloc in nc.m.functions[0].allocations:
        if not isinstance(alloc, _mb.MemoryLocationSet):
            continue
        name = alloc.memorylocations[0].name
        if alloc.kind == "ExternalInput":
            if name != partition_name:
                in_names.append(name)
        elif alloc.kind == "ExternalOutput":
            out_names.append(name)
            shape = tuple(alloc.tensor_shape)
            dtype = _mb.dt.np(alloc.dtype)
            out_avals.append(jax.core.ShapedArray(shape, dtype))
            zero_shapes.append((shape, dtype))
    n_params = len(in_names)
    n_outs = len(out_avals)
    all_in_names = list(in_names) + list(out_names)
    if partition_name is not None:
        all_in_names.append(partition_name)
    donate = tuple(range(n_params, n_params + n_outs))

    def _pjrt_body(*args):
        operands = list(args)
        if partition_name is not None:
            operands.append(b2j.partition_id_tensor())
        return tuple(b2j._bass_exec_p.bind(
            *operands, out_avals=tuple(out_avals), in_names=tuple(all_in_names),
            out_names=tuple(out_names), lowering_input_output_aliases=(),
            sim_require_finite=True, sim_require_nnan=True, nc=nc))

    devices = jax.devices()[:n_cores]
    mesh = Mesh(np.asarray(devices), ("core",))
    sharded = jax.jit(
        shard_map(_pjrt_body, mesh=mesh,
                  in_specs=(PartitionSpec("core"),) * (n_params + n_outs),
                  out_specs=(PartitionSpec("core"),) * n_outs, check_rep=False),
        donate_argnums=donate, keep_unused=True)

    def run(in_maps):
        concat_in = [
            np.concatenate([np.asarray(in_maps[c][nm]) for c in range(n_cores)], 0)
            for nm in in_names
        ]
        concat_zeros = [np.zeros((n_cores * s[0], *s[1:]), d)
                        for s, d in zero_shapes]
        out_arrs = sharded(*concat_in, *concat_zeros)
        return [
            {nm: np.asarray(out_arrs[i]).reshape(n_cores, *out_avals[i].shape)[c]
             for i, nm in enumerate(out_names)}
            for c in range(n_cores)
        ]

    _CACHE["runner"] = run
    return run
